# revision 19
# baseline (speedup 1.0000x reference)
"""MoE FFN (top-2 of 8 experts, SwiGLU) for 8 Trainium2 NeuronCores.

Strategy: expert parallelism with load-balanced expert pairing plus a
combine-weight-aware mixed-precision tier.

Routing (host, part of sharding): softmax + top-2. Experts are sorted by
routed-token count and paired big-with-small; each pair is served by two
cores and BOTH experts' token lists are split between those two cores, so
every core runs the same program shape near the average load.

Precision tier: a token-expert pair's contribution to the output is scaled
by its combine weight p, so pairs with small p tolerate more error. Each
expert's top-NB pairs by p run fully in bf16; the rest run stage A (the
x@w1 / x@w2 GEMMs, 2/3 of the FLOPs) in fp8-e4m3 with the DoubleRow perf
mode (2 contraction k-tiles per pass = 2x throughput), with exact
power-of-2 pre-scales (x*16, w*32) undone by the ACT engine's scale
(1/512) before the SwiGLU nonlinearity. Stage B stays bf16 and shares the
same w3 stream. Simulated end-to-end relmax error ~1.6e-2 at NB=1664
(gate: 2e-2); bf16-only is ~3.8e-3. The bf16 segment sizes are exactly
equal across cores (top-NB is NB for every expert here), so the fp8
segments absorb all routing imbalance at 2/3 cost.

Per-core device program per segment (expert e), with nht = H/128 h-tiles,
f-chunks of FCH columns (NFT f-tiles each):
  g_T[f, t] = sum_i w1[h_i, f]^T @ x_T[h_i, t]        (PSUM accum over h-tiles)
  u_T[f, t] likewise with w2
  h_T[f, t] = silu(g_T + b1) * (u_T + b2)             (ACT + DVE, -> bf16)
  y_T[h, t] = sum_f w3[f, h]^T @ h_T[f, t] + b3       (PSUM accum per f-chunk,
                                                       accumulated in SBUF f32)
Weights stream through SBUF one f-chunk at a time (double-buffered per
(tensor, segment) tag); tokens/outputs are SBUF-resident. Every matmul has
a 128-row stationary operand in natural layout and a [128, block] moving
operand, so the PE runs back-to-back at stream rate. The first processed
block is the small fp8 block of segment a, so the PE starts after ~0.5 MB
of DMA; the last processed block is the small fp8 block of segment b, so
the final y write-back tail is short.
"""

import numpy as np
import ml_dtypes

E = 8       # experts
K = 2       # top-k
H = 1024    # hidden
F = 4096    # ffn dim
FCH = 256   # f-chunk size (weight streaming granularity); FCH % 128 == 0
NB = 1664   # per-expert count of top-p pairs computed fully in bf16

NHT = H // 128    # h-tiles
NHP = NHT // 2    # h-tile pairs (fp8 DoubleRow)
NFCH = F // FCH   # f-chunks
NFT = FCH // 128  # f-tiles per chunk

_BF16 = ml_dtypes.bfloat16
_F8 = ml_dtypes.float8_e4m3  # TRN e4m3 (max 240)

_kernel_cache: dict[object, object] = {}
_last_in_maps = None


def _split_sizes(n: int, first_small: bool):
    """Split n (multiple of 8) into as few blocks as possible, each 256..512
    cols (a single smaller block only if n < 256): fewer blocks mean fewer
    matmul instructions per column, and >=256 moving columns keep the PE at
    stream rate (the 128-cycle stationary load stays hidden)."""
    sizes = []
    while n > 1024:
        sizes.append(512)
        n -= 512
    if n > 512:
        a = (n // 2 + 7) // 8 * 8
        sizes.extend([a, n - a])
    elif n:
        sizes.append(n)
    sizes.sort(reverse=True)
    if first_small:
        sizes = sizes[::-1]
    return sizes


def _r8(n):
    return -(-n // 8) * 8



def _layout(layA, layB):
    """Per-block descriptors in processing order: (prec, xoff, goff, sz).
    prec: "b" (bf16) or "q" (fp8 stage A); xoff: token offset inside xT
    (b16) or xq (f8) column space; goff: token offset in the global y
    layout. Segment a leads with its fp8 block (small, fast PE start);
    segment b sandwiches its fp8 block mid-segment so two small blocks are
    never adjacent across the f-chunk boundary (small blocks leave the
    DVE's y-accumulate no time to drain, stalling the PE on PSUM buffers).
    """
    blocks = {"a": [], "b": []}
    xoff_b = xoff_q = goff = 0
    for s, lay in (("a", layA), ("b", layB)):
        b16_sizes, f8_sizes = lay
        fq, bb = [], []
        for sz in f8_sizes:
            fq.append(("q", xoff_q, sz))
            xoff_q += sz
        for sz in b16_sizes:
            bb.append(("b", xoff_b, sz))
            xoff_b += sz
        if s == "a":
            order = fq + bb
        else:
            order = bb[:1] + fq + bb[1:] if bb else fq
        for prec, xo, sz in order:
            blocks[s].append((prec, xo, goff, sz))
            goff += sz
    return blocks


def _build(layA, layB, use_b2: bool):
    """Build the per-core Bass/Tile program. layX = (b16_sizes, f8_sizes)."""
    import concourse.bass as bass  # noqa: F401
    import concourse.tile as tile
    from concourse import bacc, mybir

    bf16 = mybir.dt.bfloat16
    f8 = mybir.dt.float8e4
    f32 = mybir.dt.float32
    AF = mybir.ActivationFunctionType
    DR = mybir.MatmulPerfMode.DoubleRow

    nb = {s: sum(lay[0]) for s, lay in (("a", layA), ("b", layB))}
    sf = {s: sum(lay[1]) for s, lay in (("a", layA), ("b", layB))}
    nbt, sft = nb["a"] + nb["b"], sf["a"] + sf["b"]
    caps = nbt + sft
    has_f8 = sft > 0

    # Global token-column order (y layout) follows processing order:
    # a.f8, a.b16, b.b16, b.f8. xT carries b16 x columns (a then b); xq
    # carries f8 columns (a then b).
    nc = bacc.Bacc("TRN2", target_bir_lowering=False, debug=False, num_devices=E)

    xT = nc.declare_dram_parameter("xT", [128, NHT * nbt], bf16, isOutput=False)
    if has_f8:
        xq = nc.declare_dram_parameter("xq", [128, NHT * sft], f8,
                                       isOutput=False)
    wd = {}
    for s in "ab":
        for t in ("w1", "w2", "w3"):
            wd[t + s] = nc.declare_dram_parameter(
                t + s, [NFCH, 128, NFT * H], bf16, isOutput=False
            )
        if sf[s]:
            for t in ("w1q", "w2q"):
                wd[t + s] = nc.declare_dram_parameter(
                    t + s, [NFCH, 128, NFT * H], f8, isOutput=False
                )
    bd = {}
    for s in "ab":
        bd["b1" + s] = nc.declare_dram_parameter(
            "b1" + s, [128, F // 128], f32, isOutput=False
        )
        bd["b3" + s] = nc.declare_dram_parameter(
            "b3" + s, [128, NHT], f32, isOutput=False
        )
        if use_b2:
            bd["b2" + s] = nc.declare_dram_parameter(
                "b2" + s, [128, F // 128], f32, isOutput=False
            )
    yT = nc.declare_dram_parameter("yT", [128, NHT * caps], f32, isOutput=True)

    blocks = _layout(layA, layB)

    with tile.TileContext(nc) as tc:
        with (
            tc.tile_pool(name="xp", bufs=1) as xp,
            tc.tile_pool(name="yp", bufs=1) as yp,
            tc.tile_pool(name="wp", bufs=2) as wp,
            tc.tile_pool(name="hp", bufs=2) as hp,
            tc.tile_pool(name="sp", bufs=2) as sp,
            tc.tile_pool(name="bp", bufs=1) as bp,
            tc.tile_pool(name="pg", bufs=2, space="PSUM") as pg,
            tc.tile_pool(name="pu", bufs=2, space="PSUM") as pu,
            tc.tile_pool(name="py", bufs=3, space="PSUM") as py,
        ):
            # Tokens (resident): xall bf16 block-major; per-f8-block 4D tiles
            # [128, NHP, 2, sz] so DoubleRow takes [128, 2, sz] moving APs.
            xall = xp.tile([128, max(NHT * nbt, 2)], bf16, name="xall")
            xqt = {}
            for s in "ab":
                for prec, xo, goff_, sz in blocks[s]:
                    if prec == "q":
                        xqt[(s, xo)] = xp.tile([128, NHP, 2, sz], f8,
                                               tag=f"xq{s}{xo}", name="xqb")

            # Output accumulator (resident, f32), i-major columns.
            yall = yp.tile([128, NHT * caps], f32, name="yall")

            bt = {}
            wtiles = {}

            def load_chunk(s, fc):
                # At fc 0 the PE is waiting on these bytes: issue per-j
                # slices so each matmul group unblocks after ~256KB.
                jsl = [slice(j * H, (j + 1) * H) for j in range(NFT)] \
                    if fc == 0 else [slice(None)]
                qsl = [slice(j * NHP, (j + 1) * NHP) for j in range(NFT)] \
                    if fc == 0 else [slice(None)]
                t = {}
                if sf[s]:
                    w1qc = wp.tile([128, NFT * NHP, 2, 128], f8,
                                   tag="w1q" + s, name="w1qc")
                    w2qc = wp.tile([128, NFT * NHP, 2, 128], f8,
                                   tag="w2q" + s, name="w2qc")
                    for j, sl in enumerate(qsl):
                        nc.sync.dma_start(w1qc[:, sl],
                                          wd["w1q" + s][fc][:, jsl[j]])
                        nc.scalar.dma_start(w2qc[:, sl],
                                            wd["w2q" + s][fc][:, jsl[j]])
                    t["w1q"], t["w2q"] = w1qc, w2qc
                w1c = wp.tile([128, NFT * H], bf16, tag="w1" + s, name="w1c")
                w2c = wp.tile([128, NFT * H], bf16, tag="w2" + s, name="w2c")
                w3c = wp.tile([128, NFT * H], bf16, tag="w3" + s, name="w3c")
                for sl in jsl:
                    nc.sync.dma_start(w1c[:, sl], wd["w1" + s][fc][:, sl])
                    nc.scalar.dma_start(w2c[:, sl], wd["w2" + s][fc][:, sl])
                nc.sync.dma_start(w3c[:], wd["w3" + s][fc])
                t["w1"], t["w2"], t["w3"] = w1c, w2c, w3c
                return t

            # Prologue. The first processed block's x leads the scalar
            # queue, then segment a's chunk 0 (fp8 pieces lead the sync
            # queue), biases, remaining x blocks in consumption order,
            # then segment b's chunk 0.
            prec0, xo0, _, sz0 = blocks["a"][0]
            if prec0 == "q":
                nc.scalar.dma_start(xqt[("a", xo0)][:],
                                    xq[:, NHT * xo0:NHT * (xo0 + sz0)])
            else:
                nc.scalar.dma_start(xall[:, NHT * xo0:NHT * (xo0 + sz0)],
                                    xT[:, NHT * xo0:NHT * (xo0 + sz0)])
            wtiles["a"] = load_chunk("a", 0)
            b1t = bp.tile([128, F // 128], f32, tag="b1a", name="b1t")
            nc.sync.dma_start(b1t[:], bd["b1a"][:])
            bt["b1a"] = b1t
            for s in "ab":
                for prec, xo, goff_, sz in blocks[s]:
                    if s == "a" and xo == xo0 and prec == prec0:
                        continue
                    if prec == "q":
                        xqb = xqt[(s, xo)]
                        nc.gpsimd.dma_start(
                            xqb[:, :NHP // 2],
                            xq[:, NHT * xo:NHT * xo + NHT // 2 * sz])
                        nc.gpsimd.dma_start(
                            xqb[:, NHP // 2:],
                            xq[:, NHT * xo + NHT // 2 * sz:NHT * (xo + sz)])
                    else:
                        lo, hi = NHT * xo, NHT * (xo + sz)
                        mid = (lo + hi) // 2
                        nc.gpsimd.dma_start(xall[:, lo:mid], xT[:, lo:mid])
                        nc.gpsimd.dma_start(xall[:, mid:hi], xT[:, mid:hi])
            b3t = bp.tile([128, NHT], f32, tag="b3a", name="b3t")
            nc.sync.dma_start(b3t[:], bd["b3a"][:])
            bt["b3a"] = b3t
            if use_b2:
                b2t = bp.tile([128, F // 128], f32, tag="b2a", name="b2t")
                nc.sync.dma_start(b2t[:], bd["b2a"][:])
                bt["b2a"] = b2t
            wtiles["b"] = load_chunk("b", 0)
            for nm in ("b1b", "b3b") + (("b2b",) if use_b2 else ()):
                sz_ = F // 128 if nm[1] in "12" else NHT
                t_ = bp.tile([128, sz_], f32, tag=nm, name="bxt")
                nc.sync.dma_start(t_[:], bd[nm][:])
                bt[nm] = t_

            def stage_b(w3c, b3t_, goff, sz, ht, fc):
                # y_T[h, tok] += w3_chunk.T @ h_T ; w3c cols: (j, h).
                for i in range(NHT):
                    psy = py.tile([128, sz], f32, tag="y", name="psy",
                                  padded_shape=[128, 512])
                    for j in range(NFT):
                        nc.tensor.matmul(
                            psy[:],
                            w3c[:, j * H + i * 128:j * H + (i + 1) * 128],
                            ht[:, j * sz:(j + 1) * sz],
                            start=(j == 0), stop=(j == NFT - 1),
                        )
                    lo = i * caps + goff
                    dst = yall[:, lo:lo + sz]
                    if fc == 0:
                        nc.scalar.activation(
                            dst, psy[:], AF.Identity, bias=b3t_[:, i:i + 1]
                        )
                    else:
                        nc.vector.tensor_add(dst, dst, psy[:])
                    if fc == NFCH - 1:
                        nc.sync.dma_start(yT[:, lo:lo + sz], dst)

            pending = None
            for fc in range(NFCH):
                for s in "ab":
                    wt = wtiles[s] if fc == 0 else load_chunk(s, fc)
                    b1t = bt["b1" + s]
                    blist = blocks[s]
                    if fc == NFCH - 1 and s == "b":
                        # end on the smallest block: shortest add+DMA tail
                        blist = sorted(blist, key=lambda b: -b[3])
                    for prec, xo, goff, sz in blist:
                        ht = hp.tile([128, NFT * sz], bf16, tag="h", name="ht",
                                     padded_shape=[128, NFT * 512])
                        for j in range(NFT):
                            fg = fc * NFT + j
                            psg = pg.tile([128, sz], f32, tag="g", name="psg",
                                          padded_shape=[128, 512])
                            psu = pu.tile([128, sz], f32, tag="u", name="psu",
                                          padded_shape=[128, 512])
                            stile = sp.tile([128, sz], f32, tag="s",
                                            name="stile",
                                            padded_shape=[128, 512])
                            if prec == "q":
                                xb = xqt[(s, xo)]
                                for i2 in range(NHP):
                                    nc.tensor.matmul(
                                        psg[:], wt["w1q"][:, j * NHP + i2],
                                        xb[:, i2],
                                        start=(i2 == 0), stop=(i2 == NHP - 1),
                                        perf_mode=DR,
                                    )
                                nc.scalar.activation(
                                    stile[:], psg[:], AF.Silu,
                                    bias=b1t[:, fg:fg + 1], scale=1.0 / 512,
                                )
                                for i2 in range(NHP):
                                    nc.tensor.matmul(
                                        psu[:], wt["w2q"][:, j * NHP + i2],
                                        xb[:, i2],
                                        start=(i2 == 0), stop=(i2 == NHP - 1),
                                        perf_mode=DR,
                                    )
                                u2 = sp.tile([128, sz], f32, tag="u2",
                                             name="u2tile",
                                             padded_shape=[128, 512])
                                nc.scalar.activation(
                                    u2[:], psu[:], AF.Identity,
                                    bias=(bt["b2" + s][:, fg:fg + 1]
                                          if use_b2 else 0.0),
                                    scale=1.0 / 512,
                                )
                                nc.vector.tensor_mul(
                                    ht[:, j * sz:(j + 1) * sz], stile[:], u2[:]
                                )
                            else:
                                xcol = NHT * xo
                                for i in range(NHT):
                                    wb = (j * NHT + i) * 128
                                    nc.tensor.matmul(
                                        psg[:], wt["w1"][:, wb:wb + 128],
                                        xall[:, xcol + i * sz:
                                             xcol + (i + 1) * sz],
                                        start=(i == 0), stop=(i == NHT - 1),
                                    )
                                nc.scalar.activation(
                                    stile[:], psg[:], AF.Silu,
                                    bias=b1t[:, fg:fg + 1],
                                )
                                for i in range(NHT):
                                    wb = (j * NHT + i) * 128
                                    nc.tensor.matmul(
                                        psu[:], wt["w2"][:, wb:wb + 128],
                                        xall[:, xcol + i * sz:
                                             xcol + (i + 1) * sz],
                                        start=(i == 0), stop=(i == NHT - 1),
                                    )
                                hsl = ht[:, j * sz:(j + 1) * sz]
                                if use_b2:
                                    u2 = sp.tile([128, sz], f32, tag="u2",
                                                 name="u2tile",
                                                 padded_shape=[128, 512])
                                    nc.scalar.activation(
                                        u2[:], psu[:], AF.Identity,
                                        bias=bt["b2" + s][:, fg:fg + 1],
                                    )
                                    nc.vector.tensor_mul(hsl, stile[:], u2[:])
                                else:
                                    nc.vector.tensor_mul(hsl, stile[:], psu[:])

                        if pending is not None:
                            stage_b(*pending)
                        pending = (wt["w3"], bt["b3" + s], goff, sz, ht, fc)
            stage_b(*pending)

    nc.finalize()
    return nc


def _route(x2d: np.ndarray, router_w: np.ndarray):
    """Host router: softmax over experts, top-2. Returns per-expert token
    index lists (sorted by combine weight, descending) and combine weights."""
    logits = x2d @ router_w                       # [T, E]
    logits -= logits.max(axis=-1, keepdims=True)
    p = np.exp(logits, dtype=np.float32)
    p /= p.sum(axis=-1, keepdims=True)
    order = np.argsort(-p, axis=-1, kind="stable")[:, :K]   # [T, K]
    idx_e, cw_e = [], []
    for e in range(E):
        sel = np.nonzero((order == e).any(axis=1))[0]
        cw = p[sel, e]
        o = np.argsort(-cw, kind="stable")
        idx_e.append(sel[o])
        cw_e.append(cw[o])
    return idx_e, cw_e


def _pack_w12(w: np.ndarray, scale, dt) -> np.ndarray:
    """[H, F] f32 -> [NFCH, 128, NFT*H] with column order (j, i, q):
    chunk c, partition p, f-tile j, h-tile i, col q = w[i*128+p, c*FCH+j*128+q].
    """
    t = np.asarray(w, dtype=np.float32).reshape(NHT, 128, NFCH, NFT, 128)
    t = (t * scale).transpose(2, 1, 3, 0, 4)  # [c, p, j, i, q]
    return np.ascontiguousarray(t.astype(dt)).reshape(NFCH, 128, NFT * H)


def _pack_w3(w: np.ndarray) -> np.ndarray:
    """[F, H] f32 -> [NFCH, 128, NFT*H] bf16 with column order (j, h):
    chunk c, partition p (= f within f-tile j) -> w[c*FCH+j*128+p, h]."""
    t = np.asarray(w, dtype=np.float32).reshape(NFCH, NFT, 128, H)
    t = t.transpose(0, 2, 1, 3)  # [c, p, j, h]
    return np.ascontiguousarray(t.astype(_BF16)).reshape(NFCH, 128, NFT * H)


def _pack_x(rows: np.ndarray, sizes, cap, scale, dt) -> np.ndarray:
    """[n, H] f32 rows -> [128, NHT*cap] in block-major column order."""
    n = len(rows)
    xg = np.zeros((cap, H), dtype=dt)
    if n:
        xg[:n] = (rows * scale).astype(dt)
    parts, off = [], 0
    for sz in sizes:
        parts.append(xg[off:off + sz].reshape(sz, NHT, 128)
                     .transpose(2, 1, 0).reshape(128, NHT * sz))
        off += sz
    return np.concatenate(parts, axis=1) if parts else \
        np.zeros((128, 0), dtype=dt)


def kernel(x, router_w, w1, b1, w2, b2, w3, b3):
    from concourse.bass_utils import run_bass_kernel_spmd

    B, S, _ = x.shape
    T = B * S
    x2d = np.ascontiguousarray(x, dtype=np.float32).reshape(T, H)

    idx_e, cw_e = _route(x2d, np.asarray(router_w, dtype=np.float32))
    loads = np.array([len(i) for i in idx_e])
    order = np.argsort(-loads, kind="stable")
    bigs, smalls = order[:4], order[4:]
    # Per-expert split: top-NB pairs by p in bf16, rest fp8 (stage A).
    nb_e = np.minimum(loads, NB)
    nf_e = loads - nb_e
    # Per-core segment sizes (each expert splits across its pair's 2 cores).
    nb1 = max(_r8(int(-(-int(nb_e[bigs].max()) // 2))), 16)
    nb2 = max(_r8(int(-(-int(nb_e[smalls].max()) // 2))), 16)
    sf1 = _r8(int(-(-int(nf_e[bigs].max()) // 2))) if nf_e[bigs].max() else 0
    sf2 = _r8(int(-(-int(nf_e[smalls].max()) // 2))) \
        if nf_e[smalls].max() else 0
    layA = (_split_sizes(nb1, first_small=False), _split_sizes(sf1, True))
    layB = (_split_sizes(nb2, first_small=False), _split_sizes(sf2, False))

    use_b2 = bool(np.any(b2))
    key = (tuple(layA[0]), tuple(layA[1]), tuple(layB[0]), tuple(layB[1]),
           use_b2)
    nc = _kernel_cache.get(key)
    if nc is None:
        nc = _build(layA, layB, use_b2)
        _kernel_cache[key] = nc

    has_f8 = sf1 + sf2 > 0
    # Pack per-expert weights once; cores of a pair share the arrays.
    wpk = {}
    for e in range(E):
        wpk[e] = (
            _pack_w12(w1[e], 1.0, _BF16), _pack_w12(w2[e], 1.0, _BF16),
            _pack_w3(w3[e]),
            _pack_w12(w1[e], 32.0, _F8) if has_f8 else None,
            _pack_w12(w2[e], 32.0, _F8) if has_f8 else None,
        )
    bpk = {}
    for e in range(E):
        bpk[e] = (
            np.ascontiguousarray(
                np.asarray(b1[e], dtype=np.float32).reshape(F // 128, 128).T),
            np.ascontiguousarray(
                np.asarray(b3[e], dtype=np.float32).reshape(NHT, 128).T),
            np.ascontiguousarray(
                np.asarray(b2[e], dtype=np.float32).reshape(F // 128, 128).T)
            if use_b2 else None,
        )

    # Split each expert's b16/f8 token lists between its pair's two cores.
    seg_tokens = []  # per core: (eA, idxAb, cwAb, idxAf, cwAf, eB, ...)
    for pr in range(4):
        eA, eB = int(bigs[pr]), int(smalls[pr])
        for half in range(2):
            ent = []
            for e in (eA, eB):
                idx, cw = idx_e[e], cw_e[e]
                nb_, nf_ = int(nb_e[e]), int(nf_e[e])
                hb, hf = -(-nb_ // 2), -(-nf_ // 2)
                sb = slice(0, hb) if half == 0 else slice(hb, nb_)
                sq = slice(nb_, nb_ + hf) if half == 0 else \
                    slice(nb_ + hf, None)
                ent.extend([e, idx[sb], cw[sb], idx[sq], cw[sq]])
            seg_tokens.append(tuple(ent))

    in_maps = []
    for c in range(E):
        eA, iAb, cAb, iAf, cAf, eB, iBb, cBb, iBf, cBf = seg_tokens[c]
        m = {}
        m["xT"] = np.ascontiguousarray(np.concatenate(
            [_pack_x(x2d[iAb], layA[0], nb1, 1.0, _BF16),
             _pack_x(x2d[iBb], layB[0], nb2, 1.0, _BF16)], axis=1))
        if has_f8:
            m["xq"] = np.ascontiguousarray(np.concatenate(
                [_pack_x(x2d[iAf], layA[1], sf1, 16.0, _F8),
                 _pack_x(x2d[iBf], layB[1], sf2, 16.0, _F8)], axis=1))
        for s, e, sfx in (("a", eA, sf1), ("b", eB, sf2)):
            m["w1" + s], m["w2" + s], m["w3" + s] = wpk[e][:3]
            if sfx:
                m["w1q" + s], m["w2q" + s] = wpk[e][3], wpk[e][4]
            m["b1" + s], m["b3" + s] = bpk[e][0], bpk[e][1]
            if use_b2:
                m["b2" + s] = bpk[e][2]
        in_maps.append(m)

    global _last_in_maps
    _last_in_maps = in_maps
    res = run_bass_kernel_spmd(nc, in_maps, core_ids=list(range(E)))

    # Map each block's y columns back to its token sub-range.
    blocks = _layout(layA, layB)
    out = np.zeros((T, H), dtype=np.float32)
    for c in range(E):
        eA, iAb, cAb, iAf, cAf, eB, iBb, cBb, iBf, cBf = seg_tokens[c]
        yTe = res.results[c]["yT"].reshape(128, NHT, -1)
        toks = {("a", "b"): (iAb, cAb), ("a", "q"): (iAf, cAf),
                ("b", "b"): (iBb, cBb), ("b", "q"): (iBf, cBf)}
        # xoff is relative to the per-core (segment, prec) list start.
        base = {("a", "b"): 0, ("a", "q"): 0,
                ("b", "b"): nb1, ("b", "q"): sf1}
        for s in "ab":
            for prec, xo, goff, sz in blocks[s]:
                idx, cw = toks[(s, prec)]
                lo = xo - base[(s, prec)]
                sub = idx[lo:lo + sz]
                n = len(sub)
                if n:
                    ye = yTe[:, :, goff:goff + n].transpose(2, 1, 0)
                    out[sub] += ye.reshape(n, H) * cw[lo:lo + n, None]
    return out.reshape(B, S, H)
